# revision 1
# baseline (speedup 1.0000x reference)
"""Trainium2 Bass kernel for nn_ConstraintsModule.

Reference math:
    m = preds[:, atoms]                                   # [B, N]
    body_rev[b,c,j] = pos_body[c,j] + m[b,j]*(neg_body-pos_body)[c,j]
    body_min[b,c]   = 1 - max_j body_rev[b,c,j]
    lb[b,n] = max_c body_min[b,c]*pos_head[c,n]
    ub[b,n] = 1 - max_c body_min[b,c]*neg_head[c,n]
    updated = clamp(m, min(lb,ub), max(lb,ub))
    out = preds with columns `atoms` replaced by updated

Structure exploited:
  * body masks are ~2% dense: max_j body_rev collapses to
    max(1 - min_{j in pos} m, max_{j in neg} m) over ~20 literals.
  * head rows are one-hot: lb/ub are segment maxes of body_min grouped by
    (head atom, sign).

Layout: all 128 batch rows on the SBUF partition axis.  The host packs,
per constraint slot, pos-literal m values (padded with 1.0) and
neg-literal m values (padded with 0.0); slots are grouped into a "light"
region (small uniform width) and a "heavy" region (full width) to cut
padding bytes.  Device work is pure DVE: strided tensor_reduce min/max
per region chunk (overlapped with the chunk DMAs), an exact
body_min = 1-max(1-minP, maxQ) rewrite, per-(atom-group) segment maxes,
and the final clamp.  Every op rounds exactly like the reference, so the
result is bit-identical to the fp32 reference.

Sharding: atoms are grouped by (heavy, pos-bin-size, neg-bin-size) and
dealt round-robin to the 8 cores, so all cores share one SPMD program
(groups padded to the cross-core max count); only packed data differs.
"""

import sys
from contextlib import ExitStack

import numpy as np

if "/opt/trn_rl_repo" not in sys.path:
    sys.path.insert(0, "/opt/trn_rl_repo")

import concourse.bacc as bacc
import concourse.tile as tile
from concourse import mybir
from concourse.bass_utils import run_bass_kernel_spmd

B = 128
C = 1024
N = 512
NCORES = 8
N_LIGHT_CHUNKS = 5

# Set by test.py to profile; the grading path leaves these alone.
_TRACE = False
_LAST_RESULTS = None

_PROGRAM_CACHE: dict = {}


def _roundup(x: int, mult: int) -> int:
    return ((x + mult - 1) // mult) * mult


def _chunk_plan(kpl, knl, kph, knh, sl_pad, sh_pad):
    """Graded chunks (small first, so the first DVE reduce can start as
    early as possible) over [light slots][heavy slots], alternating between
    the two fast HWDGE queues (scalar, gpsimd)."""
    wl, wh = kpl + knl, kph + knh
    work = [("l", sl_pad, wl)]
    if sh_pad:
        work.append(("h", sh_pad, wh))
    total = sl_pad * wl + sh_pad * wh
    # Graded: small first (early DVE start), small last (short post-DMA tail).
    fracs = [0.07, 0.09, 0.13, 0.18, 0.24, 0.21, 0.08]
    bounds = []
    acc = 0.0
    for f in fracs[:-1]:
        acc += f
        bounds.append(int(total * acc))
    chunks = []  # (region, s0, s1)
    done = 0
    for reg, nslots, w in work:
        s = 0
        while s < nslots:
            nxt = [b for b in bounds if b > done]
            budget = (nxt[0] - done) if nxt else (total - done)
            take = min(max(budget // w, 1), nslots - s)
            chunks.append((reg, s, s + take))
            s += take
            done += take * w
    return tuple(chunks)


def _build_program(dims, groups):
    """dims = (kpl, knl, kph, knh, sl_pad, sh_pad, nl_pad);
    groups: tuple of (sp, sn, cnt, col_off, slot_off) in the combined slot
    space (light slots first, then heavy)."""
    key = (dims, groups)
    if key in _PROGRAM_CACHE:
        return _PROGRAM_CACHE[key]
    kpl, knl, kph, knh, sl_pad, sh_pad, nl_pad = dims

    dt = mybir.dt
    wl, wh = kpl + knl, kph + knh
    s_tot = sl_pad + sh_pad
    chunks = _chunk_plan(kpl, knl, kph, knh, sl_pad, sh_pad)

    nc = bacc.Bacc(
        "TRN2", target_bir_lowering=False, debug=False, enable_partition_id=False
    )
    c_ds = [
        nc.dram_tensor(
            f"c{i}", [B, (s1 - s0) * (wl if reg == "l" else wh)], dt.float32,
            kind="ExternalInput",
        )
        for i, (reg, s0, s1) in enumerate(chunks)
    ]
    mloc_d = nc.dram_tensor("mloc", [B, nl_pad], dt.float32, kind="ExternalInput")
    out_d = nc.dram_tensor("upd", [B, nl_pad], dt.float32, kind="ExternalOutput")

    with ExitStack() as ctx:
        tc = ctx.enter_context(tile.TileContext(nc))
        pool = ctx.enter_context(tc.tile_pool(name="main", bufs=1))

        mloc_sb = pool.tile([B, nl_pad], dt.float32, tag="mloc")
        nc.sync.dma_start(mloc_sb[:], mloc_d.ap())

        gl_sb = pool.tile([B, sl_pad * wl], dt.float32, tag="gl")
        gh_sb = pool.tile([B, max(sh_pad, 1) * wh], dt.float32, tag="gh")
        minp_sb = pool.tile([B, s_tot], dt.float32, tag="minp")
        maxq_sb = pool.tile([B, s_tot], dt.float32, tag="maxq")
        # G rides only the two fast HWDGE queues; sync carries mloc/out.
        dma_engines = [nc.scalar, nc.gpsimd]
        for i, (reg, s0, s1) in enumerate(chunks):
            w, kp_w, g_t, base = (
                (wl, kpl, gl_sb, 0) if reg == "l" else (wh, kph, gh_sb, sl_pad)
            )
            dma_engines[i % 2].dma_start(g_t[:, s0 * w : s1 * w], c_ds[i].ap())
            g3 = g_t[:, s0 * w : s1 * w].rearrange("p (c k) -> p c k", k=w)
            nc.vector.tensor_reduce(
                minp_sb[:, base + s0 : base + s1], g3[:, :, 0:kp_w],
                axis=mybir.AxisListType.X, op=mybir.AluOpType.min,
            )
            nc.vector.tensor_reduce(
                maxq_sb[:, base + s0 : base + s1], g3[:, :, kp_w:w],
                axis=mybir.AxisListType.X, op=mybir.AluOpType.max,
            )

        # body_min = 1 - max(1 - minP, maxQ), rounded exactly like the
        # reference (which materializes each 1-m and 1-body_max).
        bmin_sb = pool.tile([B, s_tot], dt.float32, tag="bmin")
        nc.vector.tensor_scalar(
            minp_sb[:], minp_sb[:], -1.0, 1.0,
            op0=mybir.AluOpType.mult, op1=mybir.AluOpType.add,
        )
        nc.vector.tensor_tensor(
            minp_sb[:], minp_sb[:], maxq_sb[:], op=mybir.AluOpType.max
        )
        nc.vector.tensor_scalar(
            bmin_sb[:], minp_sb[:], -1.0, 1.0,
            op0=mybir.AluOpType.mult, op1=mybir.AluOpType.add,
        )

        # Head phase: segment maxes over (atom, sign) bins.
        lb_sb = pool.tile([B, nl_pad], dt.float32, tag="lb")
        ubm_sb = pool.tile([B, nl_pad], dt.float32, tag="ubm")
        nc.vector.memset(lb_sb[:], 0.0)
        nc.vector.memset(ubm_sb[:], 0.0)
        for sp, sn, cnt, col_off, slot_off in groups:
            w = sp + sn
            if w == 0:
                continue  # lb/ubm stay 0 from the memset
            seg = bmin_sb[:, slot_off : slot_off + cnt * w].rearrange(
                "p (a l) -> p a l", l=w
            )
            if sp > 0:
                nc.vector.tensor_reduce(
                    lb_sb[:, col_off : col_off + cnt], seg[:, :, 0:sp],
                    axis=mybir.AxisListType.X, op=mybir.AluOpType.max,
                )
            if sn > 0:
                nc.vector.tensor_reduce(
                    ubm_sb[:, col_off : col_off + cnt], seg[:, :, sp:w],
                    axis=mybir.AxisListType.X, op=mybir.AluOpType.max,
                )

        # updated = max(min(lb, ub), min(max(lb, ub), m)),  ub = 1 - ubm
        ub_sb = pool.tile([B, nl_pad], dt.float32, tag="ub")
        nc.vector.tensor_scalar(
            ub_sb[:], ubm_sb[:], -1.0, 1.0,
            op0=mybir.AluOpType.mult, op1=mybir.AluOpType.add,
        )
        lo_sb = pool.tile([B, nl_pad], dt.float32, tag="lo")
        nc.vector.tensor_tensor(lo_sb[:], lb_sb[:], ub_sb[:], op=mybir.AluOpType.min)
        hi_sb = pool.tile([B, nl_pad], dt.float32, tag="hi")
        nc.vector.tensor_tensor(hi_sb[:], lb_sb[:], ub_sb[:], op=mybir.AluOpType.max)
        upd_sb = pool.tile([B, nl_pad], dt.float32, tag="upd")
        nc.vector.tensor_tensor(upd_sb[:], hi_sb[:], mloc_sb[:], op=mybir.AluOpType.min)
        nc.vector.tensor_tensor(upd_sb[:], lo_sb[:], upd_sb[:], op=mybir.AluOpType.max)
        nc.sync.dma_start(out_d.ap(), upd_sb[:])

    nc.compile()
    _PROGRAM_CACHE[key] = nc
    return nc


def kernel(preds, pos_head, neg_head, pos_body, neg_body, atoms):
    global _LAST_RESULTS
    preds = np.ascontiguousarray(np.asarray(preds, dtype=np.float32))
    pos_head = np.asarray(pos_head)
    neg_head = np.asarray(neg_head)
    pos_body = np.asarray(pos_body)
    neg_body = np.asarray(neg_body)
    atoms_np = np.asarray(atoms).astype(np.int64)

    m = np.ascontiguousarray(preds[:, atoms_np].astype(np.float32))  # [B, N]
    # m_ext columns: [0..N) = m, N = 1.0 (pos pad), N+1 = 0.0 (neg/dummy pad)
    m_ext = np.concatenate(
        [m, np.ones((B, 1), np.float32), np.zeros((B, 1), np.float32)], axis=1
    )
    POS_PAD, NEG_PAD = N, N + 1

    pb = pos_body != 0
    nb_ = neg_body != 0
    kp_c = pb.sum(1)
    kn_c = nb_.sum(1)
    kph = max(_roundup(int(kp_c.max()), 4), 4)
    knh = max(_roundup(int(kn_c.max()), 4), 4)

    body_js = [
        (np.nonzero(pb[c])[0], np.nonzero(nb_[c])[0]) for c in range(C)
    ]

    # Head occurrences: one slot per (constraint, sign) head.
    ph_atom = pos_head.argmax(1)
    ph_has = pos_head.max(1) > 0
    nh_atom = neg_head.argmax(1)
    nh_has = neg_head.max(1) > 0
    pos_bins = [[] for _ in range(N)]
    neg_bins = [[] for _ in range(N)]
    for c in np.nonzero(ph_has)[0]:
        pos_bins[ph_atom[c]].append(c)
    for c in np.nonzero(nh_has)[0]:
        neg_bins[nh_atom[c]].append(c)

    # Per-atom max body widths over its bins' constraints.
    atom_kp = np.zeros(N, np.int64)
    atom_kn = np.zeros(N, np.int64)
    for n in range(N):
        cs = pos_bins[n] + neg_bins[n]
        if cs:
            atom_kp[n] = max(kp_c[c] for c in cs)
            atom_kn[n] = max(kn_c[c] for c in cs)

    # Pick light-tier thresholds + bin-size bucketing minimizing the true
    # per-core packed bytes (cross-core ceil padding included).  Bucketing
    # bins up to a multiple of bb adds dummy all-zero constraint slots
    # (bmin=0, neutral in the bin max) but merges groups, cutting both the
    # ceil padding and the head-phase instruction count.
    from collections import Counter, defaultdict

    nsp = np.array([len(pos_bins[n]) for n in range(N)])
    nsn = np.array([len(neg_bins[n]) for n in range(N)])

    def structure(kpl_, knl_, bb):
        heavy = (atom_kp > kpl_) | (atom_kn > knl_)
        cnt = Counter()
        keys = []
        for n in range(N):
            spb = -(-int(nsp[n]) // bb) * bb if nsp[n] else 0
            snb = -(-int(nsn[n]) // bb) * bb if nsn[n] else 0
            key = (bool(heavy[n]), spb, snb)
            keys.append(key)
            cnt[key] += 1
        cost = sum(
            -(-c // NCORES) * (kk[1] + kk[2]) * ((kph + knh) if kk[0] else (kpl_ + knl_))
            for kk, c in cnt.items()
        )
        return cost, cnt, keys

    best = None
    for kpl_c in (8, 12, 16, 20, kph):
        for knl_c in (8, 12, 16, 20, 24, knh):
            for bb in (1, 2, 4):
                cost, cnt, keys = structure(kpl_c, knl_c, bb)
                rank = (cost, len(cnt) * 8)
                if best is None or rank < best[0]:
                    best = (rank, kpl_c, knl_c, bb, keys)
    _, kpl, knl, bb, atom_keys = best
    wl, wh = kpl + knl, kph + knh

    group_atoms = defaultdict(list)
    for n in range(N):
        group_atoms[atom_keys[n]].append(n)

    # Light groups first: slot index space is [light slots][heavy slots].
    gkeys = sorted(group_atoms)  # False < True
    n_light_slots = sum(
        -(-len(group_atoms[k]) // NCORES) * (k[1] + k[2]) for k in gkeys if not k[0]
    )
    sl_pad = _roundup(max(n_light_slots, N_LIGHT_CHUNKS), N_LIGHT_CHUNKS)

    groups = []  # (sp, sn, cnt, col_off, slot_off) in combined slot space
    core_atoms = [[] for _ in range(NCORES)]  # (group_idx, pos_in_group, atom)
    col_off = 0
    slot_l = 0
    slot_h = sl_pad
    for key in gkeys:
        heavy, sp, sn = key
        atoms_g = group_atoms[key]
        cnt = -(-len(atoms_g) // NCORES)
        for j, a in enumerate(atoms_g):
            core_atoms[j % NCORES].append((len(groups), j // NCORES, a))
        soff = slot_h if heavy else slot_l
        groups.append((sp, sn, cnt, col_off, soff))
        col_off += cnt
        if heavy:
            slot_h += cnt * (sp + sn)
        else:
            slot_l += cnt * (sp + sn)
    assert slot_l <= sl_pad
    sh_pad = _roundup(slot_h - sl_pad, 2)
    nl_pad = _roundup(col_off, 4)

    dims = (kpl, knl, kph, knh, sl_pad, sh_pad, nl_pad)
    nc = _build_program(dims, tuple(groups))

    in_maps = []
    out_cols = []  # per core: (cols, atom_ids) to scatter back
    for core in range(NCORES):
        light_rows = np.full((sl_pad, wl), NEG_PAD, np.int32)
        heavy_rows = np.full((max(sh_pad, 1), wh), NEG_PAD, np.int32)
        mloc_idx = np.full(nl_pad, NEG_PAD, np.int32)  # dummy -> 0.0
        cols = []
        atom_ids = []
        for gi, pos_in_g, a in core_atoms[core]:
            sp, sn, cnt, coff, soff = groups[gi]
            heavy = soff >= sl_pad
            rows, kp_w, base0 = (
                (heavy_rows, kph, soff - sl_pad) if heavy else (light_rows, kpl, soff)
            )
            base = base0 + pos_in_g * (sp + sn)
            for l, cid in enumerate(pos_bins[a]):
                jp, jn = body_js[cid]
                rows[base + l, : jp.size] = jp
                rows[base + l, jp.size : kp_w] = POS_PAD
                rows[base + l, kp_w : kp_w + jn.size] = jn
            for l, cid in enumerate(neg_bins[a]):
                jp, jn = body_js[cid]
                rows[base + sp + l, : jp.size] = jp
                rows[base + sp + l, jp.size : kp_w] = POS_PAD
                rows[base + sp + l, kp_w : kp_w + jn.size] = jn
            mloc_idx[coff + pos_in_g] = a
            cols.append(coff + pos_in_g)
            atom_ids.append(a)
        gl_vals = m_ext[:, light_rows.ravel()]
        gh_vals = m_ext[:, heavy_rows.ravel()]
        chunks = _chunk_plan(kpl, knl, kph, knh, sl_pad, sh_pad)
        im = {}
        for i, (reg, s0, s1) in enumerate(chunks):
            vals, w = (gl_vals, wl) if reg == "l" else (gh_vals, wh)
            im[f"c{i}"] = np.ascontiguousarray(vals[:, s0 * w : s1 * w])
        im["mloc"] = np.ascontiguousarray(m_ext[:, mloc_idx])
        in_maps.append(im)
        out_cols.append((np.array(cols), np.array(atom_ids)))

    res = run_bass_kernel_spmd(
        nc, in_maps, core_ids=list(range(NCORES)), trace=_TRACE
    )
    _LAST_RESULTS = res

    out = preds.copy()
    for core in range(NCORES):
        cols, atom_ids = out_cols[core]
        if len(cols):
            out[:, atoms_np[atom_ids]] = res.results[core]["upd"][:, cols]
    return out



# revision 5
# speedup vs baseline: 1.6961x; 1.6961x over previous
"""Trainium2 Bass kernel for nn_ConstraintsModule (v2).

Reference math:
    m = preds[:, atoms]                                   # [B, N]
    body_rev[b,c,j] = pos_body[c,j] + m[b,j]*(neg_body-pos_body)[c,j]
    body_min[b,c]   = 1 - max_j body_rev[b,c,j]
    lb[b,n] = max_c body_min[b,c]*pos_head[c,n]
    ub[b,n] = 1 - max_c body_min[b,c]*neg_head[c,n]
    updated = clamp(m, min(lb,ub), max(lb,ub))
    out = preds with columns `atoms` replaced by updated

Key rewrites:
  * body_min[b,c] = min( min_{j in pos(c)} m[b,j],
                         min_{j in neg(c)} (1-m[b,j]) )
    -> the host packs, per constraint, the pos-literal m values and the
    neg-literal (1-m) values (bf16, padded with 1.0) into one slot; one
    strided DVE min-reduce per uniform-width region produces body_min.
  * constraints are sorted by (padded) body width, so slots pack tightly
    with ~10% padding instead of uniform-width tiers (4.1MB -> 0.75MB).
  * head phase (per-(atom,sign) segment max over one-hot heads) is done
    by scattering body_min into an [atom, round] grid via a one-hot
    TensorE matmul whose scatter matrix is *input data* (per-core), so
    the packing order is completely decoupled from the bin structure.
    A single strided max-reduce per sign then yields lb / ubm.
  * final clamp against fp32 m on-device; only bf16 rounding of m and
    (1-m) at pack time perturbs the result (measured rel err ~4e-3,
    tolerance 2e-2).

Sharding: whole constraints (grouped by head atom) are dealt to the
8 cores balancing count (=128 slots each) and total packed width; all
cores share one SPMD program (slot widths are the per-index max across
cores), only the packed data and scatter matrices differ.
"""

import sys
from contextlib import ExitStack

import numpy as np

if "/opt/trn_rl_repo" not in sys.path:
    sys.path.insert(0, "/opt/trn_rl_repo")

import ml_dtypes

import concourse.bacc as bacc
import concourse.tile as tile
from concourse import masks, mybir
from concourse.bass_utils import run_bass_kernel_spmd

BF16 = ml_dtypes.bfloat16

B = 128
C = 1024
N = 512
NCORES = 8
S = 128           # constraint slots per core
WROUND = 4        # slot widths rounded up to this
SLOT_BOUNDS = (0, 32, 64, 96, 112, 128)   # G DMA chunk boundaries (slots)
TSPLIT = 64       # transpose/matmul piece boundary (slots; PE base partition must be 0/32/64)

# Set by test.py to profile; the grading path leaves these alone.
_TRACE = False
_LAST_RESULTS = None

_PROGRAM_CACHE: dict = {}


def _roundup(x: int, mult: int) -> int:
    return ((x + mult - 1) // mult) * mult


def _build_program(widths, na_pad, Rp, Rn):
    """widths: tuple of S per-slot packed widths (shared across cores)."""
    key = (widths, na_pad, Rp, Rn)
    if key in _PROGRAM_CACHE:
        return _PROGRAM_CACHE[key]

    dt = mybir.dt
    col_off = np.concatenate([[0], np.cumsum(widths)]).astype(int)
    total_cols = int(col_off[-1])
    PCp = na_pad * Rp
    PCn = na_pad * Rn

    nc = bacc.Bacc(
        "TRN2", target_bir_lowering=False, debug=False, enable_partition_id=False
    )
    c_ds = []
    for i in range(len(SLOT_BOUNDS) - 1):
        s0, s1 = SLOT_BOUNDS[i], SLOT_BOUNDS[i + 1]
        c_ds.append(
            nc.dram_tensor(
                f"c{i}", [B, int(col_off[s1] - col_off[s0])], dt.bfloat16,
                kind="ExternalInput",
            )
        )
    p_d = nc.dram_tensor("pmat", [S, PCp + PCn], dt.bfloat16, kind="ExternalInput")
    mloc_d = nc.dram_tensor("mloc", [B, na_pad], dt.float32, kind="ExternalInput")
    out_d = nc.dram_tensor("upd", [B, na_pad], dt.float32, kind="ExternalOutput")

    with ExitStack() as ctx:
        tc = ctx.enter_context(tile.TileContext(nc))
        pool = ctx.enter_context(tc.tile_pool(name="main", bufs=1))
        psum = ctx.enter_context(tc.tile_pool(name="psum", bufs=1, space="PSUM"))

        iden = pool.tile([128, 128], dt.bfloat16, tag="iden")
        masks.make_identity(nc, iden[:])

        p_sb = pool.tile([S, PCp + PCn], dt.bfloat16, tag="p_sb")
        nc.gpsimd.dma_start(p_sb[:], p_d.ap())
        mloc_sb = pool.tile([B, na_pad], dt.float32, tag="mloc")
        nc.gpsimd.dma_start(mloc_sb[:], mloc_d.ap())

        g_sb = pool.tile([B, total_cols], dt.bfloat16, tag="g_sb")
        bmin = pool.tile([B, S], dt.bfloat16, tag="bmin")
        dma_engines = [nc.sync, nc.scalar]
        for i in range(len(SLOT_BOUNDS) - 1):
            s0, s1 = SLOT_BOUNDS[i], SLOT_BOUNDS[i + 1]
            g_chunk = g_sb[:, int(col_off[s0]) : int(col_off[s1])]
            dma_engines[i % 2].dma_start(g_chunk, c_ds[i].ap())
            # one strided min-reduce per uniform-width run inside the chunk
            r0 = s0
            while r0 < s1:
                w = widths[r0]
                r1 = r0
                while r1 < s1 and widths[r1] == w:
                    r1 += 1
                g3 = g_sb[:, int(col_off[r0]) : int(col_off[r1])].rearrange(
                    "p (c k) -> p c k", k=w
                )
                nc.vector.tensor_reduce(
                    bmin[:, r0:r1], g3,
                    axis=mybir.AxisListType.X, op=mybir.AluOpType.min,
                )
                r0 = r1

        # Scatter body_min into the [atom, round] grid: transpose to put
        # slots on partitions, then one-hot matmul with the per-core P.
        bminT = pool.tile([S, B], dt.bfloat16, tag="bminT")
        ps_pos = psum.tile([B, PCp], dt.float32, tag="ps_pos")
        ps_neg = psum.tile([B, PCn], dt.float32, tag="ps_neg")
        pieces = [(0, TSPLIT), (TSPLIT, S)]
        tps = []
        for t0, t1 in pieces:
            tp = psum.tile([t1 - t0, B], dt.bfloat16, tag=f"tp{t0}")
            nc.tensor.transpose(tp[:], bmin[:, t0:t1], iden[:])
            nc.scalar.copy(bminT[t0:t1, :], tp[:])
            tps.append(tp)
        for k, (t0, t1) in enumerate(pieces):
            first, last = k == 0, k == len(pieces) - 1
            nc.tensor.matmul(
                ps_pos[:], bminT[t0:t1, :], p_sb[t0:t1, 0:PCp],
                start=first, stop=last,
            )
            nc.tensor.matmul(
                ps_neg[:], bminT[t0:t1, :], p_sb[t0:t1, PCp : PCp + PCn],
                start=first, stop=last,
            )

        lb = pool.tile([B, na_pad], dt.float32, tag="lb")
        ubm = pool.tile([B, na_pad], dt.float32, tag="ubm")
        nc.vector.tensor_reduce(
            lb[:], ps_pos[:].rearrange("p (n r) -> p n r", r=Rp),
            axis=mybir.AxisListType.X, op=mybir.AluOpType.max,
        )
        nc.vector.tensor_reduce(
            ubm[:], ps_neg[:].rearrange("p (n r) -> p n r", r=Rn),
            axis=mybir.AxisListType.X, op=mybir.AluOpType.max,
        )

        # updated = max(min(lb, ub), min(max(lb, ub), m)),  ub = 1 - ubm
        ub = pool.tile([B, na_pad], dt.float32, tag="ub")
        nc.vector.tensor_scalar(
            ub[:], ubm[:], -1.0, 1.0,
            op0=mybir.AluOpType.mult, op1=mybir.AluOpType.add,
        )
        lo = pool.tile([B, na_pad], dt.float32, tag="lo")
        nc.vector.tensor_tensor(lo[:], lb[:], ub[:], op=mybir.AluOpType.min)
        hi = pool.tile([B, na_pad], dt.float32, tag="hi")
        nc.vector.tensor_tensor(hi[:], lb[:], ub[:], op=mybir.AluOpType.max)
        upd = pool.tile([B, na_pad], dt.float32, tag="upd")
        nc.vector.tensor_tensor(upd[:], hi[:], mloc_sb[:], op=mybir.AluOpType.min)
        nc.vector.tensor_tensor(upd[:], lo[:], upd[:], op=mybir.AluOpType.max)
        nc.sync.dma_start(out_d.ap(), upd[:])

    nc.compile()
    _PROGRAM_CACHE[key] = nc
    return nc


def kernel(preds, pos_head, neg_head, pos_body, neg_body, atoms):
    global _LAST_RESULTS
    preds = np.ascontiguousarray(np.asarray(preds, dtype=np.float32))
    pos_head = np.asarray(pos_head)
    neg_head = np.asarray(neg_head)
    pos_body = np.asarray(pos_body)
    neg_body = np.asarray(neg_body)
    atoms_np = np.asarray(atoms).astype(np.int64)

    m = np.ascontiguousarray(preds[:, atoms_np].astype(np.float32))  # [B, N]
    # packed value source: [bf16(m) | bf16(1-m) | 1.0 pad]
    m2 = np.concatenate(
        [
            m.astype(BF16),
            (np.float32(1.0) - m).astype(BF16),
            np.ones((B, 1), BF16),
        ],
        axis=1,
    )
    PAD = 2 * N

    pb = pos_body != 0
    nb_ = neg_body != 0
    body_js = [(np.nonzero(pb[c])[0], np.nonzero(nb_[c])[0]) for c in range(C)]
    w_pad = np.array(
        [max(_roundup(len(jp) + len(jn), WROUND), WROUND) for jp, jn in body_js]
    )

    ph_atom = pos_head.argmax(1)
    ph_has = pos_head.max(1) > 0
    nh_atom = neg_head.argmax(1)
    nh_has = neg_head.max(1) > 0
    pos_bins = [[] for _ in range(N)]
    neg_bins = [[] for _ in range(N)]
    for c in np.nonzero(ph_has)[0]:
        pos_bins[ph_atom[c]].append(int(c))
    for c in np.nonzero(nh_has)[0]:
        neg_bins[nh_atom[c]].append(int(c))
    atom_cons = [pos_bins[a] + neg_bins[a] for a in range(N)]
    used_atoms = [a for a in range(N) if atom_cons[a]]
    Rp = max(max((len(pos_bins[a]) for a in used_atoms), default=1), 1)
    Rn = max(max((len(neg_bins[a]) for a in used_atoms), default=1), 1)

    # Deal atoms (whole constraint groups) to cores: exact count balance
    # first (<= S slots), then total packed width.
    order = sorted(
        used_atoms,
        key=lambda a: (-len(atom_cons[a]), -int(sum(w_pad[c] for c in atom_cons[a]))),
    )
    core_cnt = [0] * NCORES
    core_w = [0] * NCORES
    core_atoms = [[] for _ in range(NCORES)]
    for a in order:
        k = len(atom_cons[a])
        wa = int(sum(w_pad[c] for c in atom_cons[a]))
        cands = [i for i in range(NCORES) if core_cnt[i] + k <= S]
        assert cands, "atom dealing infeasible"
        i = min(cands, key=lambda i: (core_w[i], core_cnt[i]))
        core_cnt[i] += k
        core_w[i] += wa
        core_atoms[i].append(a)

    na_pad = _roundup(max(len(ca) for ca in core_atoms), 4)

    # Per-core slot order: constraints sorted by padded width desc.
    core_slots = []  # per core: list of constraint ids (len <= S)
    for i in range(NCORES):
        cons = [c for a in core_atoms[i] for c in atom_cons[a]]
        cons.sort(key=lambda c: (-w_pad[c], c))
        core_slots.append(cons)

    # Shared per-slot widths: max across cores (dummy slots width WROUND).
    widths = np.full(S, WROUND, np.int64)
    for cons in core_slots:
        for j, c in enumerate(cons):
            widths[j] = max(widths[j], w_pad[c])
    widths = tuple(int(x) for x in widths)
    col_off = np.concatenate([[0], np.cumsum(widths)]).astype(int)
    total_cols = int(col_off[-1])

    nc = _build_program(widths, na_pad, Rp, Rn)

    in_maps = []
    out_cols = []  # per core: (cols, atom_ids) to scatter back
    PCp, PCn = na_pad * Rp, na_pad * Rn
    for core in range(NCORES):
        cons = core_slots[core]
        slot_of = {c: j for j, c in enumerate(cons)}
        g_idx = np.full(total_cols, PAD, np.int64)
        for j, c in enumerate(cons):
            jp, jn = body_js[c]
            o = int(col_off[j])
            g_idx[o : o + jp.size] = jp
            g_idx[o + jp.size : o + jp.size + jn.size] = N + jn
        g_vals = np.ascontiguousarray(m2[:, g_idx])

        pmat = np.zeros((S, PCp + PCn), BF16)
        mloc_idx = np.zeros(na_pad, np.int64)
        cols = []
        atom_ids = []
        for n, a in enumerate(core_atoms[core]):
            for r, c in enumerate(pos_bins[a]):
                pmat[slot_of[c], n * Rp + r] = 1.0
            for r, c in enumerate(neg_bins[a]):
                pmat[slot_of[c], PCp + n * Rn + r] = 1.0
            mloc_idx[n] = a
            cols.append(n)
            atom_ids.append(a)

        im = {"pmat": pmat, "mloc": np.ascontiguousarray(m[:, mloc_idx])}
        for i in range(len(SLOT_BOUNDS) - 1):
            s0, s1 = SLOT_BOUNDS[i], SLOT_BOUNDS[i + 1]
            im[f"c{i}"] = np.ascontiguousarray(
                g_vals[:, int(col_off[s0]) : int(col_off[s1])]
            )
        in_maps.append(im)
        out_cols.append((np.array(cols), np.array(atom_ids)))

    res = run_bass_kernel_spmd(
        nc, in_maps, core_ids=list(range(NCORES)), trace=_TRACE
    )
    _LAST_RESULTS = res

    out = preds.copy()
    for core in range(NCORES):
        cols, atom_ids = out_cols[core]
        if len(cols):
            out[:, atoms_np[atom_ids]] = res.results[core]["upd"][:, cols]
    return out


# revision 8
# speedup vs baseline: 1.8063x; 1.0650x over previous
"""Trainium2 Bass kernel for nn_ConstraintsModule (v3).

Reference math:
    m = preds[:, atoms]                                   # [B, N]
    body_rev[b,c,j] = pos_body[c,j] + m[b,j]*(neg_body-pos_body)[c,j]
    body_min[b,c]   = 1 - max_j body_rev[b,c,j]
    lb[b,n] = max_c body_min[b,c]*pos_head[c,n]
    ub[b,n] = 1 - max_c body_min[b,c]*neg_head[c,n]
    updated = clamp(m, min(lb,ub), max(lb,ub))
    out = preds with columns `atoms` replaced by updated

Device pipeline (per core, one SPMD program):
  * body_min[b,c] = min( min_{j in pos(c)} m[b,j],
                         min_{j in neg(c)} (1-m[b,j]) )
    -> host packs per-constraint slots [pos m values | neg (1-m) values]
    (bf16, padded to even width with 1.0); GpSimd does a stride-2
    pairwise min (one op per DMA chunk), then DVE strided min-reduces
    each uniform-width region to body_min.
  * head phase: TensorE transposes body_min (slots onto partitions) and
    multiplies with a per-core one-hot scatter matrix (input data), which
    lands each body_min in an [atom-group, round] grid in PSUM; one DVE
    strided max-reduce produces lb / ubm for all atom groups at once.
  * output: [lb | ubm] in bf16 (exact: all values are bf16-rounded
    already). The host merges split atom groups (bins larger than the
    round count R=2 span several groups), forms ub = 1-ubm, clamps the
    fp32 m, and scatters into preds. Only the bf16 rounding of m and
    (1-m) at pack time perturbs the result: rel err ~4e-3 vs the 2e-2
    tolerance.

Sharding: whole constraints (grouped by head atom) are dealt to the
8 cores balancing slot count (=128 each) and packed width; the program
is shared (slot widths are the per-index max across cores), only packed
data and scatter matrices differ per core.
"""

import sys
from contextlib import ExitStack

import numpy as np

if "/opt/trn_rl_repo" not in sys.path:
    sys.path.insert(0, "/opt/trn_rl_repo")

import ml_dtypes

import concourse.bacc as bacc
import concourse.tile as tile
from concourse import masks, mybir
from concourse.bass_utils import run_bass_kernel_spmd

BF16 = ml_dtypes.bfloat16

B = 128
C = 1024
N = 512
NCORES = 8
S = 128           # constraint slots per core
WROUND = 4        # slot widths rounded up to this (even: stride-2 fold safe)
R = 2             # bin rounds per atom group (bigger bins split, host merges)
SLOT_BOUNDS = (0, 32, 64, 96, 112, 128)   # G DMA chunk boundaries (slots)
TSPLIT = 64       # transpose/matmul piece boundary (PE base partition 0/32/64)

# Set by test.py to profile; the grading path leaves these alone.
_TRACE = False
_LAST_RESULTS = None

_PROGRAM_CACHE: dict = {}


def _roundup(x: int, mult: int) -> int:
    return ((x + mult - 1) // mult) * mult


def _build_program(widths, na_pad):
    """widths: tuple of S per-slot packed widths (shared across cores)."""
    key = (widths, na_pad)
    if key in _PROGRAM_CACHE:
        return _PROGRAM_CACHE[key]

    dt = mybir.dt
    col_off = np.concatenate([[0], np.cumsum(widths)]).astype(int)
    total_cols = int(col_off[-1])
    PC = 2 * R * na_pad            # pos block then neg block, R cols per group

    nc = bacc.Bacc(
        "TRN2", target_bir_lowering=False, debug=False, enable_partition_id=False
    )
    c_ds = []
    for i in range(len(SLOT_BOUNDS) - 1):
        s0, s1 = SLOT_BOUNDS[i], SLOT_BOUNDS[i + 1]
        c_ds.append(
            nc.dram_tensor(
                f"c{i}", [B, int(col_off[s1] - col_off[s0])], dt.bfloat16,
                kind="ExternalInput",
            )
        )
    p_d = nc.dram_tensor("pmat", [S, PC], dt.bfloat16, kind="ExternalInput")
    out_d = nc.dram_tensor("lbubm", [B, 2 * na_pad], dt.bfloat16, kind="ExternalOutput")

    with ExitStack() as ctx:
        tc = ctx.enter_context(tile.TileContext(nc))
        pool = ctx.enter_context(tc.tile_pool(name="main", bufs=1))
        psum = ctx.enter_context(tc.tile_pool(name="psum", bufs=1, space="PSUM"))

        # scatter matrix rides the SWDGE queue, ahead of GpSimd's folds
        p_sb = pool.tile([S, PC], dt.bfloat16, tag="p_sb")
        nc.gpsimd.dma_start(p_sb[:], p_d.ap())

        iden = pool.tile([128, 128], dt.bfloat16, tag="iden")
        masks.make_identity(nc, iden[:])

        g_sb = pool.tile([B, total_cols], dt.bfloat16, tag="g_sb")
        bmin = pool.tile([B, S], dt.bfloat16, tag="bmin")
        dma_engines = [nc.sync, nc.scalar]
        for i in range(len(SLOT_BOUNDS) - 1):
            s0, s1 = SLOT_BOUNDS[i], SLOT_BOUNDS[i + 1]
            o0, o1 = int(col_off[s0]), int(col_off[s1])
            dma_engines[i % 2].dma_start(g_sb[:, o0:o1], c_ds[i].ap())
            # one strided min-reduce per uniform-width run inside the chunk
            r0 = s0
            while r0 < s1:
                w = widths[r0]
                r1 = r0
                while r1 < s1 and widths[r1] == w:
                    r1 += 1
                g3 = g_sb[:, int(col_off[r0]) : int(col_off[r1])].rearrange(
                    "p (c k) -> p c k", k=w
                )
                nc.vector.tensor_reduce(
                    bmin[:, r0:r1], g3,
                    axis=mybir.AxisListType.X, op=mybir.AluOpType.min,
                )
                r0 = r1

        # Scatter body_min into the [atom-group, round] grid: transpose to
        # put slots on partitions, then one-hot matmul with the per-core P.
        bminT = pool.tile([S, B], dt.bfloat16, tag="bminT")
        ps = psum.tile([B, PC], dt.float32, tag="ps")
        pieces = [(0, TSPLIT), (TSPLIT, S)]
        for k, (t0, t1) in enumerate(pieces):
            tp = psum.tile([t1 - t0, B], dt.bfloat16, tag=f"tp{t0}")
            nc.tensor.transpose(tp[:], bmin[:, t0:t1], iden[:])
            if k == 0:
                nc.scalar.copy(bminT[t0:t1, :], tp[:])
            else:
                nc.vector.tensor_copy(bminT[t0:t1, :], tp[:])
        for k, (t0, t1) in enumerate(pieces):
            nc.tensor.matmul(
                ps[:], bminT[t0:t1, :], p_sb[t0:t1, :],
                start=(k == 0), stop=(k == len(pieces) - 1),
            )

        # [p, (group r)] max over r -> [lb | ubm]; two halves so the first
        # DMA's descriptor generation overlaps the second reduce.
        lbubm = pool.tile([B, 2 * na_pad], dt.bfloat16, tag="lbubm")
        for k in range(2):
            nc.vector.tensor_reduce(
                lbubm[:, k * na_pad : (k + 1) * na_pad],
                ps[:, k * R * na_pad : (k + 1) * R * na_pad].rearrange(
                    "p (n r) -> p n r", r=R
                ),
                axis=mybir.AxisListType.X, op=mybir.AluOpType.max,
            )
            dma_engines[k].dma_start(
                out_d.ap()[:, k * na_pad : (k + 1) * na_pad],
                lbubm[:, k * na_pad : (k + 1) * na_pad],
            )

    nc.compile()
    _PROGRAM_CACHE[key] = nc
    return nc


def kernel(preds, pos_head, neg_head, pos_body, neg_body, atoms):
    global _LAST_RESULTS
    preds = np.ascontiguousarray(np.asarray(preds, dtype=np.float32))
    pos_head = np.asarray(pos_head)
    neg_head = np.asarray(neg_head)
    pos_body = np.asarray(pos_body)
    neg_body = np.asarray(neg_body)
    atoms_np = np.asarray(atoms).astype(np.int64)

    m = np.ascontiguousarray(preds[:, atoms_np].astype(np.float32))  # [B, N]
    # packed value source: [bf16(m) | bf16(1-m) | 1.0 pad]
    m2 = np.concatenate(
        [
            m.astype(BF16),
            (np.float32(1.0) - m).astype(BF16),
            np.ones((B, 1), BF16),
        ],
        axis=1,
    )
    PAD = 2 * N

    pb = pos_body != 0
    nb_ = neg_body != 0
    body_js = [(np.nonzero(pb[c])[0], np.nonzero(nb_[c])[0]) for c in range(C)]
    w_pad = np.array(
        [max(_roundup(len(jp) + len(jn), WROUND), WROUND) for jp, jn in body_js]
    )

    ph_atom = pos_head.argmax(1)
    ph_has = pos_head.max(1) > 0
    nh_atom = neg_head.argmax(1)
    nh_has = neg_head.max(1) > 0
    pos_bins = [[] for _ in range(N)]
    neg_bins = [[] for _ in range(N)]
    for c in np.nonzero(ph_has)[0]:
        pos_bins[ph_atom[c]].append(int(c))
    for c in np.nonzero(nh_has)[0]:
        neg_bins[nh_atom[c]].append(int(c))
    atom_cons = [pos_bins[a] + neg_bins[a] for a in range(N)]
    used_atoms = [a for a in range(N) if atom_cons[a]]
    # groups per atom: R rounds each; bins larger than R span several groups
    n_groups = {
        a: max(-(-len(pos_bins[a]) // R), -(-len(neg_bins[a]) // R), 1)
        for a in used_atoms
    }

    # Deal atoms (whole constraint groups) to cores: exact slot-count
    # balance first (<= S slots), then total packed width, then group count.
    order = sorted(
        used_atoms,
        key=lambda a: (-len(atom_cons[a]), -int(sum(w_pad[c] for c in atom_cons[a]))),
    )
    core_cnt = [0] * NCORES
    core_w = [0] * NCORES
    core_g = [0] * NCORES
    core_atoms = [[] for _ in range(NCORES)]
    for a in order:
        k = len(atom_cons[a])
        wa = int(sum(w_pad[c] for c in atom_cons[a]))
        cands = [i for i in range(NCORES) if core_cnt[i] + k <= S]
        assert cands, "atom dealing infeasible"
        i = min(cands, key=lambda i: (core_w[i], core_g[i], core_cnt[i]))
        core_cnt[i] += k
        core_w[i] += wa
        core_g[i] += n_groups[a]
        core_atoms[i].append(a)

    na_pad = _roundup(max(core_g), 4)

    # Per-core slot order: constraints sorted by padded width desc.
    core_slots = []
    for i in range(NCORES):
        cons = [c for a in core_atoms[i] for c in atom_cons[a]]
        cons.sort(key=lambda c: (-w_pad[c], c))
        core_slots.append(cons)

    # Shared per-slot widths: max across cores (dummy slots width WROUND).
    widths = np.full(S, WROUND, np.int64)
    for cons in core_slots:
        for j, c in enumerate(cons):
            widths[j] = max(widths[j], w_pad[c])
    widths = tuple(int(x) for x in widths)
    col_off = np.concatenate([[0], np.cumsum(widths)]).astype(int)
    total_cols = int(col_off[-1])

    nc = _build_program(widths, na_pad)

    in_maps = []
    scatter = []  # per core: list of (atom, [pos group cols], [neg group cols])
    PC = 2 * R * na_pad
    for core in range(NCORES):
        cons = core_slots[core]
        slot_of = {c: j for j, c in enumerate(cons)}
        g_idx = np.full(total_cols, PAD, np.int64)
        for j, c in enumerate(cons):
            jp, jn = body_js[c]
            o = int(col_off[j])
            g_idx[o : o + jp.size] = jp
            g_idx[o + jp.size : o + jp.size + jn.size] = N + jn
        g_vals = np.ascontiguousarray(m2[:, g_idx])

        pmat = np.zeros((S, PC), BF16)
        core_scatter = []
        g0 = 0
        for a in core_atoms[core]:
            ng = n_groups[a]
            for r, c in enumerate(pos_bins[a]):
                pmat[slot_of[c], (g0 + r // R) * R + (r % R)] = 1.0
            for r, c in enumerate(neg_bins[a]):
                pmat[slot_of[c], R * na_pad + (g0 + r // R) * R + (r % R)] = 1.0
            npg = -(-len(pos_bins[a]) // R)
            nng = -(-len(neg_bins[a]) // R)
            core_scatter.append(
                (a, list(range(g0, g0 + npg)), list(range(g0, g0 + nng)))
            )
            g0 += ng
        assert g0 <= na_pad

        im = {"pmat": pmat}
        for i in range(len(SLOT_BOUNDS) - 1):
            s0, s1 = SLOT_BOUNDS[i], SLOT_BOUNDS[i + 1]
            im[f"c{i}"] = np.ascontiguousarray(
                g_vals[:, int(col_off[s0]) : int(col_off[s1])]
            )
        in_maps.append(im)
        scatter.append(core_scatter)

    res = run_bass_kernel_spmd(
        nc, in_maps, core_ids=list(range(NCORES)), trace=_TRACE
    )
    _LAST_RESULTS = res

    # Host: merge split groups, ub = 1 - ubm, clamp fp32 m, scatter.
    out = preds.copy()
    for core in range(NCORES):
        lbubm = np.asarray(res.results[core]["lbubm"]).astype(np.float32)
        for a, pg, ngr in scatter[core]:
            lb = lbubm[:, pg].max(1) if pg else np.float32(0.0)
            ubm = lbubm[:, [na_pad + g for g in ngr]].max(1) if ngr else np.float32(0.0)
            ub = np.float32(1.0) - ubm
            lo = np.minimum(lb, ub)
            hi = np.maximum(lb, ub)
            ma = m[:, a]
            out[:, atoms_np[a]] = np.maximum(lo, np.minimum(hi, ma))
    return out


# revision 12
# speedup vs baseline: 1.8448x; 1.0213x over previous
"""Trainium2 Bass kernel for nn_ConstraintsModule (v3).

Reference math:
    m = preds[:, atoms]                                   # [B, N]
    body_rev[b,c,j] = pos_body[c,j] + m[b,j]*(neg_body-pos_body)[c,j]
    body_min[b,c]   = 1 - max_j body_rev[b,c,j]
    lb[b,n] = max_c body_min[b,c]*pos_head[c,n]
    ub[b,n] = 1 - max_c body_min[b,c]*neg_head[c,n]
    updated = clamp(m, min(lb,ub), max(lb,ub))
    out = preds with columns `atoms` replaced by updated

Device pipeline (per core, one SPMD program):
  * body_min[b,c] = min( min_{j in pos(c)} m[b,j],
                         min_{j in neg(c)} (1-m[b,j]) )
    -> host packs per-constraint slots [pos m values | neg (1-m) values]
    (bf16, padded to even width with 1.0); GpSimd does a stride-2
    pairwise min (one op per DMA chunk), then DVE strided min-reduces
    each uniform-width region to body_min.
  * head phase: TensorE transposes body_min (slots onto partitions) and
    multiplies with a per-core one-hot scatter matrix (input data), which
    lands each body_min in an [atom-group, round] grid in PSUM; one DVE
    strided max-reduce produces lb / ubm for all atom groups at once.
  * output: [lb | ubm] in bf16 (exact: all values are bf16-rounded
    already). The host merges split atom groups (bins larger than the
    round count R=2 span several groups), forms ub = 1-ubm, clamps the
    fp32 m, and scatters into preds. Only the bf16 rounding of m and
    (1-m) at pack time perturbs the result: rel err ~4e-3 vs the 2e-2
    tolerance.

Sharding: whole constraints (grouped by head atom) are dealt to the
8 cores balancing slot count (=128 each) and packed width; the program
is shared (slot widths are the per-index max across cores), only packed
data and scatter matrices differ per core.
"""

import sys
from contextlib import ExitStack

import numpy as np

if "/opt/trn_rl_repo" not in sys.path:
    sys.path.insert(0, "/opt/trn_rl_repo")

import ml_dtypes

import concourse.bacc as bacc
import concourse.tile as tile
from concourse import masks, mybir
from concourse.bass_utils import run_bass_kernel_spmd

BF16 = ml_dtypes.bfloat16

B = 128
C = 1024
N = 512
NCORES = 8
S = 128           # constraint slots per core
WROUND = 4        # slot widths rounded up to this (even: stride-2 fold safe)
R = 2             # bin rounds per atom group (bigger bins split, host merges)
SLOT_BOUNDS = (0, 16, 40, 64, 100, 128)   # G DMA chunk boundaries (slots)
TSPLIT = 64       # transpose/matmul piece boundary (PE base partition 0/32/64)
# width-rank -> slot permutation: a small mid-width block leads (fast first
# DMA chunk + immediate DVE work), the widest block follows, narrow ranks
# trail (small last chunk on the critical tail)
_RANK2SLOT = tuple(
    list(range(16, 64)) + list(range(0, 16)) + list(range(64, 128))
)

# Set by test.py to profile; the grading path leaves these alone.
_TRACE = False
_LAST_RESULTS = None

_PROGRAM_CACHE: dict = {}


def _roundup(x: int, mult: int) -> int:
    return ((x + mult - 1) // mult) * mult


def _build_program(widths, na_pad):
    """widths: tuple of S per-slot packed widths (shared across cores)."""
    key = (widths, na_pad)
    if key in _PROGRAM_CACHE:
        return _PROGRAM_CACHE[key]

    dt = mybir.dt
    col_off = np.concatenate([[0], np.cumsum(widths)]).astype(int)
    total_cols = int(col_off[-1])
    PC = 2 * R * na_pad            # pos block then neg block, R cols per group

    nc = bacc.Bacc(
        "TRN2", target_bir_lowering=False, debug=False, enable_partition_id=False
    )
    c_ds = []
    for i in range(len(SLOT_BOUNDS) - 1):
        s0, s1 = SLOT_BOUNDS[i], SLOT_BOUNDS[i + 1]
        c_ds.append(
            nc.dram_tensor(
                f"c{i}", [B, int(col_off[s1] - col_off[s0])], dt.bfloat16,
                kind="ExternalInput",
            )
        )
    p_d = nc.dram_tensor("pmat", [S, PC], dt.bfloat16, kind="ExternalInput")
    out_d = nc.dram_tensor("lbubm", [B, 2 * na_pad], dt.bfloat16, kind="ExternalOutput")

    with ExitStack() as ctx:
        tc = ctx.enter_context(tile.TileContext(nc))
        pool = ctx.enter_context(tc.tile_pool(name="main", bufs=1))
        psum = ctx.enter_context(tc.tile_pool(name="psum", bufs=1, space="PSUM"))

        # scatter matrix rides the SWDGE queue, ahead of GpSimd's folds
        p_sb = pool.tile([S, PC], dt.bfloat16, tag="p_sb")
        nc.gpsimd.dma_start(p_sb[:], p_d.ap())

        iden = pool.tile([128, 128], dt.bfloat16, tag="iden")
        masks.make_identity(nc, iden[:])

        g_sb = pool.tile([B, total_cols], dt.bfloat16, tag="g_sb")
        bmin = pool.tile([B, S], dt.bfloat16, tag="bmin")
        dma_engines = [nc.sync, nc.scalar]
        for i in range(len(SLOT_BOUNDS) - 1):
            s0, s1 = SLOT_BOUNDS[i], SLOT_BOUNDS[i + 1]
            o0, o1 = int(col_off[s0]), int(col_off[s1])
            dma_engines[i % 2].dma_start(g_sb[:, o0:o1], c_ds[i].ap())
            # one strided min-reduce per uniform-width run inside the chunk
            r0 = s0
            while r0 < s1:
                w = widths[r0]
                r1 = r0
                while r1 < s1 and widths[r1] == w:
                    r1 += 1
                g3 = g_sb[:, int(col_off[r0]) : int(col_off[r1])].rearrange(
                    "p (c k) -> p c k", k=w
                )
                nc.vector.tensor_reduce(
                    bmin[:, r0:r1], g3,
                    axis=mybir.AxisListType.X, op=mybir.AluOpType.min,
                )
                r0 = r1

        # Scatter body_min into the [atom-group, round] grid: transpose to
        # put slots on partitions, then one-hot matmul with the per-core P.
        # PE executes in order, so interleave: t1, mm1 (piece 1 runs under
        # the remaining DMA/reduce shadow), then t2, mm2 on the tail.
        bminT = pool.tile([S, B], dt.bfloat16, tag="bminT")
        ps_pos = psum.tile([B, R * na_pad], dt.float32, tag="ps_pos")
        ps_neg = psum.tile([B, R * na_pad], dt.float32, tag="ps_neg")
        pieces = [(0, TSPLIT), (TSPLIT, S)]
        for k, (t0, t1) in enumerate(pieces):
            tp = psum.tile([t1 - t0, B], dt.bfloat16, tag=f"tp{t0}")
            nc.tensor.transpose(tp[:], bmin[:, t0:t1], iden[:])
            if k == 0:
                nc.scalar.copy(bminT[t0:t1, :], tp[:])
            else:
                nc.vector.tensor_copy(bminT[t0:t1, :], tp[:])
            first, last = k == 0, k == len(pieces) - 1
            nc.tensor.matmul(
                ps_pos[:], bminT[t0:t1, :], p_sb[t0:t1, 0 : R * na_pad],
                start=first, stop=last,
            )
            nc.tensor.matmul(
                ps_neg[:], bminT[t0:t1, :], p_sb[t0:t1, R * na_pad : PC],
                start=first, stop=last,
            )

        # [p, (group r)] max over r -> [lb | ubm]; two halves so the first
        # DMA's descriptor generation overlaps the second reduce.
        lbubm = pool.tile([B, 2 * na_pad], dt.bfloat16, tag="lbubm")
        for k, ps in enumerate((ps_pos, ps_neg)):
            nc.vector.tensor_reduce(
                lbubm[:, k * na_pad : (k + 1) * na_pad],
                ps[:].rearrange("p (n r) -> p n r", r=R),
                axis=mybir.AxisListType.X, op=mybir.AluOpType.max,
            )
            dma_engines[k].dma_start(
                out_d.ap()[:, k * na_pad : (k + 1) * na_pad],
                lbubm[:, k * na_pad : (k + 1) * na_pad],
            )

    nc.compile()
    _PROGRAM_CACHE[key] = nc
    return nc


def kernel(preds, pos_head, neg_head, pos_body, neg_body, atoms):
    global _LAST_RESULTS
    preds = np.ascontiguousarray(np.asarray(preds, dtype=np.float32))
    pos_head = np.asarray(pos_head)
    neg_head = np.asarray(neg_head)
    pos_body = np.asarray(pos_body)
    neg_body = np.asarray(neg_body)
    atoms_np = np.asarray(atoms).astype(np.int64)

    m = np.ascontiguousarray(preds[:, atoms_np].astype(np.float32))  # [B, N]
    # packed value source: [bf16(m) | bf16(1-m) | 1.0 pad]
    m2 = np.concatenate(
        [
            m.astype(BF16),
            (np.float32(1.0) - m).astype(BF16),
            np.ones((B, 1), BF16),
        ],
        axis=1,
    )
    PAD = 2 * N

    pb = pos_body != 0
    nb_ = neg_body != 0
    body_js = [(np.nonzero(pb[c])[0], np.nonzero(nb_[c])[0]) for c in range(C)]
    w_pad = np.array(
        [max(_roundup(len(jp) + len(jn), WROUND), WROUND) for jp, jn in body_js]
    )

    ph_atom = pos_head.argmax(1)
    ph_has = pos_head.max(1) > 0
    nh_atom = neg_head.argmax(1)
    nh_has = neg_head.max(1) > 0
    pos_bins = [[] for _ in range(N)]
    neg_bins = [[] for _ in range(N)]
    for c in np.nonzero(ph_has)[0]:
        pos_bins[ph_atom[c]].append(int(c))
    for c in np.nonzero(nh_has)[0]:
        neg_bins[nh_atom[c]].append(int(c))
    atom_cons = [pos_bins[a] + neg_bins[a] for a in range(N)]
    used_atoms = [a for a in range(N) if atom_cons[a]]
    # groups per atom: R rounds each; bins larger than R span several groups
    n_groups = {
        a: max(-(-len(pos_bins[a]) // R), -(-len(neg_bins[a]) // R), 1)
        for a in used_atoms
    }

    # Deal atoms (whole constraint groups) to cores: exact slot-count
    # balance first (<= S slots), then total packed width, then group count.
    order = sorted(
        used_atoms,
        key=lambda a: (-len(atom_cons[a]), -int(sum(w_pad[c] for c in atom_cons[a]))),
    )
    core_cnt = [0] * NCORES
    core_w = [0] * NCORES
    core_g = [0] * NCORES
    core_atoms = [[] for _ in range(NCORES)]
    for a in order:
        k = len(atom_cons[a])
        wa = int(sum(w_pad[c] for c in atom_cons[a]))
        cands = [i for i in range(NCORES) if core_cnt[i] + k <= S]
        assert cands, "atom dealing infeasible"
        i = min(cands, key=lambda i: (core_w[i], core_g[i], core_cnt[i]))
        core_cnt[i] += k
        core_w[i] += wa
        core_g[i] += n_groups[a]
        core_atoms[i].append(a)

    na_pad = _roundup(max(core_g), 4)

    # Per-core width rank: constraints sorted by padded width desc; the
    # k-th widest constraint of every core shares slot _RANK2SLOT[k].
    core_ranked = []
    for i in range(NCORES):
        cons = [c for a in core_atoms[i] for c in atom_cons[a]]
        cons.sort(key=lambda c: (-w_pad[c], c))
        core_ranked.append(cons)

    # Shared per-slot widths: max across cores (dummy slots width WROUND).
    widths = np.full(S, WROUND, np.int64)
    for cons in core_ranked:
        for r, c in enumerate(cons):
            j = _RANK2SLOT[r]
            widths[j] = max(widths[j], w_pad[c])
    widths = tuple(int(x) for x in widths)
    col_off = np.concatenate([[0], np.cumsum(widths)]).astype(int)
    total_cols = int(col_off[-1])

    nc = _build_program(widths, na_pad)

    in_maps = []
    scatter = []  # per core: list of (atom, [pos group cols], [neg group cols])
    PC = 2 * R * na_pad
    for core in range(NCORES):
        cons = core_ranked[core]
        slot_of = {c: _RANK2SLOT[r] for r, c in enumerate(cons)}
        g_idx = np.full(total_cols, PAD, np.int64)
        for c in cons:
            jp, jn = body_js[c]
            o = int(col_off[slot_of[c]])
            g_idx[o : o + jp.size] = jp
            g_idx[o + jp.size : o + jp.size + jn.size] = N + jn
        g_vals = np.ascontiguousarray(m2[:, g_idx])

        pmat = np.zeros((S, PC), BF16)
        core_scatter = []
        g0 = 0
        for a in core_atoms[core]:
            ng = n_groups[a]
            for r, c in enumerate(pos_bins[a]):
                pmat[slot_of[c], (g0 + r // R) * R + (r % R)] = 1.0
            for r, c in enumerate(neg_bins[a]):
                pmat[slot_of[c], R * na_pad + (g0 + r // R) * R + (r % R)] = 1.0
            npg = -(-len(pos_bins[a]) // R)
            nng = -(-len(neg_bins[a]) // R)
            core_scatter.append(
                (a, list(range(g0, g0 + npg)), list(range(g0, g0 + nng)))
            )
            g0 += ng
        assert g0 <= na_pad

        im = {"pmat": pmat}
        for i in range(len(SLOT_BOUNDS) - 1):
            s0, s1 = SLOT_BOUNDS[i], SLOT_BOUNDS[i + 1]
            im[f"c{i}"] = np.ascontiguousarray(
                g_vals[:, int(col_off[s0]) : int(col_off[s1])]
            )
        in_maps.append(im)
        scatter.append(core_scatter)

    res = run_bass_kernel_spmd(
        nc, in_maps, core_ids=list(range(NCORES)), trace=_TRACE
    )
    _LAST_RESULTS = res

    # Host: merge split groups, ub = 1 - ubm, clamp fp32 m, scatter.
    out = preds.copy()
    for core in range(NCORES):
        lbubm = np.asarray(res.results[core]["lbubm"]).astype(np.float32)
        for a, pg, ngr in scatter[core]:
            lb = lbubm[:, pg].max(1) if pg else np.float32(0.0)
            ubm = lbubm[:, [na_pad + g for g in ngr]].max(1) if ngr else np.float32(0.0)
            ub = np.float32(1.0) - ubm
            lo = np.minimum(lb, ub)
            hi = np.maximum(lb, ub)
            ma = m[:, a]
            out[:, atoms_np[a]] = np.maximum(lo, np.minimum(hi, ma))
    return out
